# revision 15
# baseline (speedup 1.0000x reference)
"""DepthTransformer Trainium2 kernel.

Strategy: data-parallel over the batch dim b=6 across cores (cores 6,7
process duplicate samples; host discards). Per core, one full sample:

  xp   = silu(gn_in(W_in @ x + b_in))                      [320, 1024]
  ctxp = relu(gn_ctx(W_ctx @ ctx))                         [64, 32, 1024]
  (pos_emb folded in after transpose to pixel-major)
  Folded attention (k, v never materialized):
    qt[p,(n,j)]   = sum_c' Wqk[(n,j),c'] xp[c',p],  Wqk_n = s*Wk_n^T Wq_n
    sim[p,(n,d)]  = sum_j qt[p,(n,j)] * ctxp_t[p,(d,j)]    (DVE)
    a             = softmax_d(sim)
    ctxa[p,(n,j)] = sum_d ctxp_t[p,(j,d)] * a[p,(n,d)]     (DVE)
    out1          = Wov @ ctxa_ch,  Wov = W_out @ blockdiag(Wv)
  y1 = relu(gn1(out1)); y2 = relu(gn2(conv3x3(y1, w1)))
  y  = conv3x3(y2, w2) + x
"""

import os
import numpy as np
import ml_dtypes

import concourse.bass as bass
import concourse.bacc as bacc
import concourse.tile as tile
from concourse import mybir
from concourse.bass_utils import run_bass_kernel_spmd

F32 = mybir.dt.float32
BF16 = mybir.dt.bfloat16
AF = mybir.ActivationFunctionType
ALU = mybir.AluOpType
AX = mybir.AxisListType

HN, HD, CD, D = 8, 40, 64, 32
CH = HN * HD          # 320
NPIX = 1024           # 32*32
NT = 3                # channel tiles of 128 for 320 (128,128,64 padded to 384)
EPS = 1e-5
DEBUG = bool(int(os.environ.get("DT_DEBUG", "0")))


def _bcast(ap, axis, n):
    """Insert a step-0 broadcast dim of size n at free-dim position `axis`."""
    return ap.unsqueeze(axis).broadcast_to(
        tuple(ap.shape[:axis]) + (n,) + tuple(ap.shape[axis:]))


def build_program():
    nc = bacc.Bacc("TRN2", target_bir_lowering=False, debug=False)

    def inp(name, shape, dt=F32):
        return nc.dram_tensor(name, shape, dt, kind="ExternalInput").ap()

    x_d = inp("x", [384, NPIX])
    ctx_d = inp("ctxin", [CD, D * NPIX])
    w_in_t = inp("w_in_t", [NT, NT, 128, 128])
    b_in = inp("b_in", [384, 1])
    gin_g = inp("gin_g", [384, 1])
    gin_b = inp("gin_b", [384, 1])
    wctx_t = inp("wctx_t", [CD, CD])
    gctx_g = inp("gctx_g", [CD, 1])
    gctx_b = inp("gctx_b", [CD, 1])
    wqkt_d = inp("wqkt", [NT, 128, 512])
    pos_dj = inp("pos_dj", [1, 2048], BF16)
    wovt_d = inp("wovt", [4, NT, 128, 128], BF16)
    g1_g = inp("g1_g", [384, 1])
    g1_b = inp("g1_b", [384, 1])
    g2_g = inp("g2_g", [384, 1])
    g2_b = inp("g2_b", [384, 1])
    c1_d = inp("conv1_t", [9, NT, NT, 128, 128])
    c2_d = inp("conv2_t", [9, NT, NT, 128, 128])
    gsel_d = inp("gsel", [NT, NT, 128, 128])
    g2c_d = inp("g2ctx", [CD, CD])

    y_d = nc.dram_tensor("y", [CH, NPIX], F32, kind="ExternalOutput").ap()

    # internal DRAM scratch for ctxp (raw and activated, bf16)
    ctxp_raw_d = nc.dram_tensor("ctxp_raw", [CD, D * NPIX], BF16).ap()
    ctxp_fin_d = nc.dram_tensor("ctxp_fin", [CD, D * NPIX], BF16).ap()

    dbg = {}
    if DEBUG:
        for nm, shape, dt in [
            ("xp_dbg", [CH, NPIX], F32),
            ("ctxpf_dbg", [CD, D * NPIX], BF16),
            ("qt_dbg", [NPIX, 512], BF16),
            ("sim_dbg", [NPIX, 256], F32),
            ("a_dbg", [NPIX, 256], BF16),
            ("ctxa_dbg", [NPIX, 512], BF16),
            ("out1_dbg", [CH, NPIX], F32),
            ("y1_dbg", [CH, NPIX], F32),
            ("y2_dbg", [CH, NPIX], F32),
        ]:
            dbg[nm] = nc.dram_tensor(nm, shape, dt, kind="ExternalOutput").ap()

    with tile.TileContext(nc) as tc:
        from contextlib import ExitStack

        es = ExitStack()
        persist = es.enter_context(tc.tile_pool(name="persist", bufs=1))
        wpool = es.enter_context(tc.tile_pool(name="wpool", bufs=1))
        wstream = es.enter_context(tc.tile_pool(name="wstream", bufs=8))
        stage = es.enter_context(tc.tile_pool(name="stage", bufs=3))
        small = es.enter_context(tc.tile_pool(name="small", bufs=2))
        attn = es.enter_context(tc.tile_pool(name="attn", bufs=2))
        psum = es.enter_context(tc.tile_pool(name="psum", bufs=6, space="PSUM"))
        spsum = es.enter_context(tc.tile_pool(name="spsum", bufs=2, space="PSUM"))

        # ---------------- load persistent weights ----------------
        _wn = [0]
        def load_w(pool, src, shape, dt=F32, tag=None):
            _wn[0] += 1
            t = pool.tile(shape, dt, tag=tag or "", name=f"w{_wn[0]}")
            nc.sync.dma_start(out=t[:], in_=src)
            return t

        win_sb = [[load_w(wpool, w_in_t[k, m], [128, 128]) for m in range(NT)]
                  for k in range(NT)]
        wqkt_sb = [load_w(wpool, wqkt_d[k], [128, 512]) for k in range(NT)]
        wctx_sb = load_w(wpool, wctx_t, [CD, CD])
        wovt_sb = [[load_w(wpool, wovt_d[k, m], [128, 128], BF16) for m in range(NT)]
                   for k in range(4)]
        gsel_sb = [[load_w(wpool, gsel_d[k, m], [128, 128]) for m in range(NT)]
                   for k in range(NT)]
        g2c_sb = load_w(wpool, g2c_d, [CD, CD])
        bin_sb = [load_w(wpool, b_in[m * 128:(m + 1) * 128], [128, 1]) for m in range(NT)]

        def load_vec(src):
            return [load_w(wpool, src[m * 128:(m + 1) * 128], [128, 1]) for m in range(NT)]

        gin_g_sb, gin_b_sb = load_vec(gin_g), load_vec(gin_b)
        g1_g_sb, g1_b_sb = load_vec(g1_g), load_vec(g1_b)
        g2_g_sb, g2_b_sb = load_vec(g2_g), load_vec(g2_b)
        gctx_g_sb = load_w(wpool, gctx_g, [CD, 1])
        gctx_b_sb = load_w(wpool, gctx_b, [CD, 1])

        eps_sb = wpool.tile([128, 1], F32)
        nc.vector.memset(eps_sb[:], EPS)

        posr_dj = wpool.tile([128, 2048], BF16)
        nc.sync.dma_start(out=posr_dj[:],
                          in_=pos_dj.partition_broadcast(128).squeeze(1))

        # =========== GN stats helper (320-channel, 8 groups of 40) ===========
        def gn_affine_320(raw_tiles, gamma_sb, beta_sb, name):
            """raw_tiles: 3x [128,1024] f32 SBUF. Returns (s, t) per-tile [128,1]."""
            mv2 = []
            for t in range(NT):
                st = small.tile([128, 2, 6], F32, tag=f"{name}_st{t}", name=f"{name}_st{t}")
                for i in range(2):
                    nc.vector.bn_stats(out=st[:, i, :], in_=raw_tiles[t][:, i * 512:(i + 1) * 512])
                mv = small.tile([128, 2], F32, tag=f"{name}_mv{t}", name=f"{name}_mv{t}")
                nc.vector.bn_aggr(out=mv[:], in_=st[:])
                m2 = small.tile([128, 2], F32, tag=f"{name}_mv2{t}", name=f"{name}_m2{t}")
                nc.vector.tensor_copy(m2[:, 0:1], mv[:, 0:1])
                nc.vector.tensor_mul(m2[:, 1:2], mv[:, 0:1], mv[:, 0:1])
                nc.vector.tensor_add(m2[:, 1:2], m2[:, 1:2], mv[:, 1:2])
                mv2.append(m2)
            s_t = []
            for m in range(NT):
                gp = spsum.tile([128, 2], F32, tag="gnps")
                for k in range(NT):
                    nc.tensor.matmul(gp[:], gsel_sb[k][m][:], mv2[k][:],
                                     start=(k == 0), stop=(k == NT - 1))
                gs = small.tile([128, 2], F32, tag=f"{name}_gs{m}")
                nc.vector.tensor_copy(gs[:], gp[:])
                s = small.tile([128, 1], F32, tag=f"{name}_s{m}")
                tt = small.tile([128, 1], F32, tag=f"{name}_t{m}")
                vt = small.tile([128, 1], F32, tag=f"{name}_v")
                nc.vector.tensor_mul(vt[:], gs[:, 0:1], gs[:, 0:1])
                nc.vector.tensor_sub(vt[:], gs[:, 1:2], vt[:])
                nc.scalar.activation(out=vt[:], in_=vt[:], func=AF.Sqrt, bias=eps_sb[:, 0:1])
                nc.vector.reciprocal(out=vt[:], in_=vt[:])
                nc.vector.tensor_mul(s[:], gamma_sb[m][:], vt[:])
                nc.vector.tensor_mul(tt[:], gs[:, 0:1], s[:])
                nc.vector.tensor_sub(tt[:], beta_sb[m][:], tt[:])
                s_t.append((s, tt))
            return s_t

        # ---------------- phase 1: x load + proj_in + gn_in + silu ----------------
        x_sb = [persist.tile([128, NPIX], F32, name=f"x_{t}") for t in range(NT)]
        for t in range(NT):
            nc.sync.dma_start(out=x_sb[t][:], in_=x_d[t * 128:(t + 1) * 128, :])

        xp = [persist.tile([128, NPIX], F32, name=f"xp_{t}") for t in range(NT)]
        for m in range(NT):
            for n in range(2):
                ps = psum.tile([128, 512], F32, tag="mm")
                for k in range(NT):
                    nc.tensor.matmul(ps[:], win_sb[k][m][:], x_sb[k][:, n * 512:(n + 1) * 512],
                                     start=(k == 0), stop=(k == NT - 1))
                nc.scalar.activation(out=xp[m][:, n * 512:(n + 1) * 512], in_=ps[:],
                                     func=AF.Identity, bias=bin_sb[m][:, 0:1])
        st_in = gn_affine_320(xp, gin_g_sb, gin_b_sb, "gin")
        for m in range(NT):
            s, t = st_in[m]
            nc.scalar.activation(out=xp[m][:], in_=xp[m][:], func=AF.Silu,
                                 bias=t[:, 0:1], scale=s[:, 0:1])
        if DEBUG:
            for m in range(NT):
                hi = min(128, CH - m * 128)
                nc.sync.dma_start(out=dbg["xp_dbg"][m * 128:m * 128 + hi, :], in_=xp[m][:hi, :])

        # ---------------- phase 2: ctx proj + gn stats; raw bf16 -> DRAM ----------------
        cst = persist.tile([CD, 64, 6], F32)
        for c in range(64):
            cin = stage.tile([CD, 512], F32, tag="ctxin")
            nc.sync.dma_start(out=cin[:], in_=ctx_d[:, c * 512:(c + 1) * 512])
            ps = psum.tile([CD, 512], F32, tag="mm")
            nc.tensor.matmul(ps[:], wctx_sb[:], cin[:], start=True, stop=True)
            crawbf = stage.tile([CD, 512], BF16, tag="ctxev")
            nc.vector.tensor_copy(crawbf[:], ps[:])
            nc.vector.bn_stats(out=cst[:, c, :], in_=crawbf[:])
            nc.sync.dma_start(out=ctxp_raw_d[:, c * 512:(c + 1) * 512], in_=crawbf[:])
        cmv = small.tile([CD, 2], F32)
        nc.vector.bn_aggr(out=cmv[:], in_=cst[:])
        cmv2 = small.tile([CD, 2], F32)
        nc.vector.tensor_copy(cmv2[:, 0:1], cmv[:, 0:1])
        nc.vector.tensor_mul(cmv2[:, 1:2], cmv[:, 0:1], cmv[:, 0:1])
        nc.vector.tensor_add(cmv2[:, 1:2], cmv2[:, 1:2], cmv[:, 1:2])
        cgp = spsum.tile([CD, 2], F32, tag="gnps")
        nc.tensor.matmul(cgp[:], g2c_sb[:], cmv2[:], start=True, stop=True)
        cgs = small.tile([CD, 2], F32)
        nc.vector.tensor_copy(cgs[:], cgp[:])
        cs = small.tile([CD, 1], F32)
        ct = small.tile([CD, 1], F32)
        cv = small.tile([CD, 1], F32)
        nc.vector.tensor_mul(cv[:], cgs[:, 0:1], cgs[:, 0:1])
        nc.vector.tensor_sub(cv[:], cgs[:, 1:2], cv[:])
        nc.scalar.activation(out=cv[:], in_=cv[:], func=AF.Sqrt, bias=eps_sb[:CD, 0:1])
        nc.vector.reciprocal(out=cv[:], in_=cv[:])
        nc.vector.tensor_mul(cs[:], gctx_g_sb[:], cv[:])
        nc.vector.tensor_mul(ct[:], cgs[:, 0:1], cs[:])
        nc.vector.tensor_sub(ct[:], gctx_b_sb[:], ct[:])
        # apply relu(x*s+t) in chunks DRAM->DRAM (bf16)
        for c in range(8):
            sl = slice(c * 4096, (c + 1) * 4096)
            ca = stage.tile([CD, 4096], BF16, tag="ctxap")
            nc.sync.dma_start(out=ca[:], in_=ctxp_raw_d[:, sl])
            nc.scalar.activation(out=ca[:], in_=ca[:], func=AF.Relu,
                                 bias=ct[:, 0:1], scale=cs[:, 0:1])
            nc.sync.dma_start(out=ctxp_fin_d[:, sl], in_=ca[:])
        if DEBUG:
            for c in range(8):
                sl = slice(c * 4096, (c + 1) * 4096)
                cb = stage.tile([CD, 4096], BF16, tag="ctxdbg")
                nc.sync.dma_start(out=cb[:], in_=ctxp_fin_d[:, sl])
                nc.sync.dma_start(out=dbg["ctxpf_dbg"][:, sl], in_=cb[:])

        # ---------------- phase 3: qt (pixel-major q-tilde) ----------------
        qt = [persist.tile([128, 512], BF16, name=f"qt_{p}") for p in range(8)]
        for p in range(8):
            ps = psum.tile([128, 512], F32, tag="mm")
            for k in range(NT):
                nc.tensor.matmul(ps[:], xp[k][:, p * 128:(p + 1) * 128], wqkt_sb[k][:],
                                 start=(k == 0), stop=(k == NT - 1))
            nc.scalar.activation(out=qt[p][:], in_=ps[:], func=AF.Copy)
            if DEBUG:
                nc.sync.dma_start(out=dbg["qt_dbg"][p * 128:(p + 1) * 128, :], in_=qt[p][:])

        # ---------------- phase 4: attention per pixel-tile ----------------
        ctxa_bf = [persist.tile([128, 512], BF16, name=f"cxb_{p}") for p in range(8)]
        for p in range(8):
            tdj = attn.tile([128, 32, 64], BF16, tag="tdj")
            for d in range(D):
                src = ctxp_fin_d[:, d * NPIX + p * 128: d * NPIX + (p + 1) * 128]
                nc.sync.dma_start_transpose(out=tdj[:, d, :], in_=src)
            nc.gpsimd.tensor_add(
                tdj[:], tdj[:], posr_dj[:].rearrange("p (d j) -> p d j", j=64))

            sim = attn.tile([128, 8, 32], F32, tag="sim")
            for h in range(4):
                prod = attn.tile([128, 4096], BF16, tag="prod")
                in0 = _bcast(tdj[:], 1, 2)
                in1 = _bcast(
                    qt[p][:, h * 128:(h + 1) * 128].rearrange("p (n j) -> p n j", j=64),
                    2, 32)
                nc.vector.tensor_tensor(out=prod[:], in0=in0, in1=in1, op=ALU.mult)
                nc.vector.tensor_reduce(
                    out=sim[:, h * 2:(h + 1) * 2, :],
                    in_=prod[:].rearrange("p (nd j) -> p nd j", j=64),
                    axis=AX.X, op=ALU.add)
            if DEBUG:
                nc.sync.dma_start(out=dbg["sim_dbg"][p * 128:(p + 1) * 128, :],
                                  in_=sim[:].rearrange("p n d -> p (n d)"))
            # softmax over d
            mx = small.tile([128, 8], F32, tag="mx")
            nc.vector.tensor_reduce(out=mx[:], in_=sim[:], axis=AX.X, op=ALU.max, negate=True)
            for n in range(HN):
                nc.scalar.activation(out=sim[:, n, :], in_=sim[:, n, :], func=AF.Exp,
                                     bias=mx[:, n:n + 1])
            sm = small.tile([128, 8], F32, tag="sm")
            nc.vector.tensor_reduce(out=sm[:], in_=sim[:], axis=AX.X, op=ALU.add)
            nc.vector.reciprocal(out=sm[:], in_=sm[:])
            abf = attn.tile([128, 8, 32], BF16, tag="abf")
            nc.vector.tensor_tensor(
                out=abf[:], in0=sim[:], in1=_bcast(sm[:], 2, 32), op=ALU.mult)
            if DEBUG:
                nc.sync.dma_start(out=dbg["a_dbg"][p * 128:(p + 1) * 128, :],
                                  in_=abf[:].rearrange("p n d -> p (n d)"))
            # ctxa
            cxf = attn.tile([128, 512], F32, tag="cxf")
            for h in range(4):
                prod = attn.tile([128, 4096], BF16, tag="prod")
                in0 = _bcast(tdj[:].transpose([0, 2, 1]), 1, 2)
                in1 = _bcast(abf[:, h * 2:(h + 1) * 2, :], 2, 64)
                nc.gpsimd.tensor_tensor(out=prod[:], in0=in0, in1=in1, op=ALU.mult)
                nc.vector.tensor_reduce(
                    out=cxf[:, h * 128:(h + 1) * 128],
                    in_=prod[:].rearrange("p (nj d) -> p nj d", d=32),
                    axis=AX.X, op=ALU.add)
            nc.vector.tensor_copy(ctxa_bf[p][:], cxf[:])
            if DEBUG:
                nc.sync.dma_start(out=dbg["ctxa_dbg"][p * 128:(p + 1) * 128, :], in_=ctxa_bf[p][:])

        # ---------------- phase 5: transpose ctxa to ch-major ----------------
        ctxa_ch = [persist.tile([128, NPIX], BF16, name=f"cxc_{k}") for k in range(4)]
        for kt in range(4):
            for p in range(8):
                nc.sync.dma_start_transpose(
                    out=ctxa_ch[kt][:, p * 128:(p + 1) * 128],
                    in_=ctxa_bf[p][:, kt * 128:(kt + 1) * 128])

        # ---------------- phase 6: Wov proj -> out1; gn1+relu -> y1pad ----------------
        out1 = [persist.tile([128, NPIX], F32, tag=f"out1_{m}", name=f"out1_{m}") for m in range(NT)]
        for m in range(NT):
            for n in range(2):
                ps = psum.tile([128, 512], F32, tag="mm")
                for k in range(4):
                    nc.tensor.matmul(ps[:], wovt_sb[k][m][:],
                                     ctxa_ch[k][:, n * 512:(n + 1) * 512],
                                     start=(k == 0), stop=(k == 3))
                nc.scalar.activation(out=out1[m][:, n * 512:(n + 1) * 512], in_=ps[:], func=AF.Copy)
        if DEBUG:
            for m in range(NT):
                hi = min(128, CH - m * 128)
                nc.sync.dma_start(out=dbg["out1_dbg"][m * 128:m * 128 + hi, :], in_=out1[m][:hi, :])

        st1 = gn_affine_320(out1, g1_g_sb, g1_b_sb, "gn1")
        pad1 = [persist.tile([128, 34, 34], F32, tag=f"pad1_{m}", name=f"pad1_{m}") for m in range(NT)]
        for m in range(NT):
            nc.vector.memset(pad1[m][:], 0.0)
            s, t = st1[m]
            nc.scalar.activation(out=pad1[m][:, 1:33, 1:33],
                                 in_=out1[m][:].rearrange("p (h w) -> p h w", w=32),
                                 func=AF.Relu, bias=t[:, 0:1], scale=s[:, 0:1])
        if DEBUG:
            for m in range(NT):
                hi = min(128, CH - m * 128)
                nc.sync.dma_start(
                    out=dbg["y1_dbg"][m * 128:m * 128 + hi, :],
                    in_=pad1[m][:hi, 1:33, 1:33].rearrange("p h w -> p (h w)"))

        # ---------------- conv helper ----------------
        def conv3x3(w_d, src_pad, name):
            """9-tap conv; returns out tiles 3x[128,1024] f32."""
            out = [persist.tile([128, NPIX], F32, tag=f"out1_{m}", name=f"cv_{m}") for m in range(NT)]
            for m in range(NT):
                for n in range(2):
                    r0 = n * 16
                    ps = psum.tile([128, 512], F32, tag="mm")
                    first = True
                    for tap in range(9):
                        dy, dx = tap // 3, tap % 3
                        for k in range(NT):
                            w = wstream.tile([128, 128], F32, tag="convw")
                            nc.sync.dma_start(out=w[:], in_=w_d[tap, k, m])
                            nc.tensor.matmul(
                                ps[:], w[:],
                                src_pad[k][:, r0 + dy:r0 + dy + 16, dx:dx + 32],
                                start=first, stop=(tap == 8 and k == NT - 1))
                            first = False
                    nc.scalar.activation(out=out[m][:, n * 512:(n + 1) * 512], in_=ps[:],
                                         func=AF.Copy)
            return out

        y2 = conv3x3(c1_d, pad1, "c1")
        st2 = gn_affine_320(y2, g2_g_sb, g2_b_sb, "gn2")
        pad2 = [persist.tile([128, 34, 34], F32, tag=f"pad1_{m}", name=f"pad2_{m}") for m in range(NT)]
        for m in range(NT):
            nc.vector.memset(pad2[m][:], 0.0)
            s, t = st2[m]
            nc.scalar.activation(out=pad2[m][:, 1:33, 1:33],
                                 in_=y2[m][:].rearrange("p (h w) -> p h w", w=32),
                                 func=AF.Relu, bias=t[:, 0:1], scale=s[:, 0:1])
        if DEBUG:
            for m in range(NT):
                hi = min(128, CH - m * 128)
                nc.sync.dma_start(
                    out=dbg["y2_dbg"][m * 128:m * 128 + hi, :],
                    in_=pad2[m][:hi, 1:33, 1:33].rearrange("p h w -> p (h w)"))

        # ---------------- conv2 + residual ----------------
        for m in range(NT):
            hi = min(128, CH - m * 128)
            for n in range(2):
                r0 = n * 16
                ps = psum.tile([128, 512], F32, tag="mm")
                first = True
                for tap in range(9):
                    dy, dx = tap // 3, tap % 3
                    for k in range(NT):
                        w = wstream.tile([128, 128], F32, tag="convw")
                        nc.sync.dma_start(out=w[:], in_=c2_d[tap, k, m])
                        nc.tensor.matmul(
                            ps[:], w[:],
                            pad2[k][:, r0 + dy:r0 + dy + 16, dx:dx + 32],
                            start=first, stop=(tap == 8 and k == NT - 1))
                        first = False
                fin = stage.tile([128, 512], F32, tag="fin")
                nc.vector.tensor_add(fin[:], ps[:], x_sb[m][:, n * 512:(n + 1) * 512])
                nc.sync.dma_start(out=y_d[m * 128:m * 128 + hi, n * 512:(n + 1) * 512],
                                  in_=fin[:hi, :])
        es.close()

    nc.compile()
    return nc


_PROG = None
_LAST_RESULTS = None
_LAST_EXEC_NS = None


def _get_prog():
    global _PROG
    if _PROG is None:
        _PROG = build_program()
    return _PROG


def _prep_host(inputs):
    """Precompute folded weights; returns the common (weight) part of in_map."""
    f32 = np.float32
    bf16 = ml_dtypes.bfloat16
    w_in = np.asarray(inputs["w_in"], f32)
    wq = np.asarray(inputs["wq"], f32)
    wk = np.asarray(inputs["wk"], f32)
    wv = np.asarray(inputs["wv"], f32)
    wout = np.asarray(inputs["w_attn_out"], f32)
    pos = np.asarray(inputs["pos_emb"], f32)
    scale = HD ** -0.5

    def pad_to(a, shape):
        out = np.zeros(shape, a.dtype)
        out[tuple(slice(0, s) for s in a.shape)] = a
        return out

    def tile_km(mat_t, kt, mt):  # mat_t: [K, M] -> [kt, mt, 128, 128]
        p = pad_to(mat_t, (kt * 128, mt * 128))
        return np.ascontiguousarray(
            p.reshape(kt, 128, mt, 128).transpose(0, 2, 1, 3))

    w_in_tiles = tile_km(w_in.T, NT, NT)

    wqk = np.concatenate(
        [scale * (wk[n * HD:(n + 1) * HD, :].T @ wq[n * HD:(n + 1) * HD, :])
         for n in range(HN)], axis=0)          # [512, 320]
    wqkt = pad_to(wqk.T, (NT * 128, 512)).reshape(NT, 128, 512)

    wov = np.concatenate(
        [wout[:, n * HD:(n + 1) * HD] @ wv[n * HD:(n + 1) * HD, :]
         for n in range(HN)], axis=1)          # [320, 512]
    wov_tiles = tile_km(wov.T, 4, NT).astype(bf16)

    def conv_tiles(w):  # [o, i, 3, 3] -> [9, kt, mt, 128, 128], lhsT = w[:,:,ky,kx].T
        taps = [tile_km(np.ascontiguousarray(w[:, :, t // 3, t % 3].T), NT, NT)
                for t in range(9)]
        return np.stack(taps, axis=0)

    gsel = np.zeros((CH, CH), f32)
    for g in range(8):
        gsel[g * 40:(g + 1) * 40, g * 40:(g + 1) * 40] = 1.0 / 40
    g2ctx = np.zeros((CD, CD), f32)
    for g in range(8):
        g2ctx[g * 8:(g + 1) * 8, g * 8:(g + 1) * 8] = 1.0 / 8

    def col(v):
        return pad_to(np.asarray(v, f32).reshape(-1, 1), (384, 1))

    common = {
        "w_in_t": w_in_tiles,
        "b_in": col(inputs["b_in"]),
        "gin_g": col(inputs["gn_in_g"]), "gin_b": col(inputs["gn_in_b"]),
        "wctx_t": np.ascontiguousarray(np.asarray(inputs["w_ctx"], f32).T),
        "gctx_g": np.asarray(inputs["gn_ctx_g"], f32).reshape(CD, 1),
        "gctx_b": np.asarray(inputs["gn_ctx_b"], f32).reshape(CD, 1),
        "wqkt": wqkt,
        "pos_dj": pos.reshape(1, 2048).astype(bf16),
        "wovt": wov_tiles,
        "g1_g": col(inputs["gn1_g"]), "g1_b": col(inputs["gn1_b"]),
        "g2_g": col(inputs["gn2_g"]), "g2_b": col(inputs["gn2_b"]),
        "conv1_t": conv_tiles(np.asarray(inputs["conv1_w"], f32)),
        "conv2_t": conv_tiles(np.asarray(inputs["conv2_w"], f32)),
        "gsel": tile_km(gsel, NT, NT),
        "g2ctx": g2ctx,
    }
    return common


def kernel(**inputs):
    nc = _get_prog()
    common = _prep_host(inputs)
    x = np.asarray(inputs["x"], np.float32)      # [6, 320, 32, 32]
    ctx = np.asarray(inputs["context"], np.float32)  # [6, 64, 32, 32, 32]
    b = x.shape[0]
    in_maps = []
    for core in range(8):
        s = core if core < b else core - b
        m = dict(common)
        xs = np.zeros((384, NPIX), np.float32)
        xs[:CH] = x[s].reshape(CH, NPIX)
        m["x"] = xs
        m["ctxin"] = np.ascontiguousarray(ctx[s].reshape(CD, D * NPIX))
        in_maps.append(m)
    trace = bool(int(os.environ.get("DT_TRACE", "0")))
    kw = {}
    if trace:
        import sys
        import types
        try:
            import antenv.axon_hooks  # noqa: F401
        except ImportError:
            from trn_agent_boot.trn_boot import _ntff_profile_via_ctypes
            m = types.ModuleType("antenv.axon_hooks")
            _h = _ntff_profile_via_ctypes("/opt/axon/libaxon_pjrt.so")
            m.get_axon_ntff_profile_hook = lambda: _h
            sys.modules["antenv.axon_hooks"] = m
        kw = dict(trace=True, tmpdir=os.environ.get("DT_TRACE_DIR") or None)
    res = run_bass_kernel_spmd(nc, in_maps, list(range(8)), **kw)
    global _LAST_RESULTS, _LAST_EXEC_NS
    _LAST_RESULTS = res.results
    _LAST_EXEC_NS = res.exec_time_ns
    if trace:
        print(f"HW exec time: {res.exec_time_ns} ns")
    out = np.stack([res.results[s]["y"] for s in range(b)], axis=0)
    return out.reshape(b, CH, 32, 32).astype(np.float32)


if __name__ == "__main__":
    pass


# revision 21
# speedup vs baseline: 1.6654x; 1.6654x over previous
"""DepthTransformer Trainium2 kernel.

Strategy: data-parallel over the batch dim b=6 across cores (cores 6,7
process duplicate samples; host discards). Per core, one full sample:

  xp   = silu(gn_in(W_in @ x + b_in))                      [320, 1024]
  ctxp = relu(gn_ctx(W_ctx @ ctx))                         [64, 32, 1024]
  (pos_emb folded in after transpose to pixel-major)
  Folded attention (k, v never materialized):
    qt[p,(n,j)]   = sum_c' Wqk[(n,j),c'] xp[c',p],  Wqk_n = s*Wk_n^T Wq_n
    sim[p,(n,d)]  = sum_j qt[p,(n,j)] * ctxp_t[p,(d,j)]    (DVE)
    a             = softmax_d(sim)
    ctxa[p,(n,j)] = sum_d ctxp_t[p,(j,d)] * a[p,(n,d)]     (DVE)
    out1          = Wov @ ctxa_ch,  Wov = W_out @ blockdiag(Wv)
  y1 = relu(gn1(out1)); y2 = relu(gn2(conv3x3(y1, w1)))
  y  = conv3x3(y2, w2) + x
"""

import os
import numpy as np
import ml_dtypes

import concourse.bass as bass
import concourse.bacc as bacc
import concourse.tile as tile
from concourse import mybir
from concourse.bass_utils import run_bass_kernel_spmd

F32 = mybir.dt.float32
BF16 = mybir.dt.bfloat16
AF = mybir.ActivationFunctionType
ALU = mybir.AluOpType
AX = mybir.AxisListType

HN, HD, CD, D = 8, 40, 64, 32
CH = HN * HD          # 320
NPIX = 1024           # 32*32
NT = 3                # channel tiles of 128 for 320 (128,128,64 padded to 384)
EPS = 1e-5
DEBUG = bool(int(os.environ.get("DT_DEBUG", "0")))


def _bcast(ap, axis, n):
    """Insert a step-0 broadcast dim of size n at free-dim position `axis`."""
    return ap.unsqueeze(axis).broadcast_to(
        tuple(ap.shape[:axis]) + (n,) + tuple(ap.shape[axis:]))


def build_program():
    nc = bacc.Bacc("TRN2", target_bir_lowering=False, debug=False)

    def inp(name, shape, dt=F32):
        return nc.dram_tensor(name, shape, dt, kind="ExternalInput").ap()

    x_d = inp("x", [384, NPIX])
    xbf_d = inp("x_bf", [384, NPIX], BF16)
    ctx_d = inp("ctxin", [CD, D * NPIX], BF16)
    w_in_t = inp("w_in_t", [NT, NT, 128, 128], BF16)
    b_in = inp("b_in", [384, 1])
    gin_g = inp("gin_g", [384, 1])
    gin_b = inp("gin_b", [384, 1])
    wctx_t = inp("wctx_t", [CD, CD], BF16)
    gctx_g = inp("gctx_g", [CD, 1])
    gctx_b = inp("gctx_b", [CD, 1])
    wqkt_d = inp("wqkt", [NT, 128, 512], BF16)
    pos_dj = inp("pos_dj", [1, 2048], BF16)
    pos_jd = inp("pos_jd", [1, 2048], BF16)
    wovt_d = inp("wovt", [4, NT, 128, 128], BF16)
    g1_g = inp("g1_g", [384, 1])
    g1_b = inp("g1_b", [384, 1])
    g2_g = inp("g2_g", [384, 1])
    g2_b = inp("g2_b", [384, 1])
    c1_d = inp("conv1_t", [NT, NT, 128, 9, 128], BF16)
    c2_d = inp("conv2_t", [NT, NT, 128, 9, 128], BF16)
    gsel_d = inp("gsel", [NT, NT, 128, 128])
    g2c_d = inp("g2ctx", [CD, CD])

    y_d = nc.dram_tensor("y", [CH, NPIX], F32, kind="ExternalOutput").ap()

    # internal DRAM scratch for ctxp: raw ch-major; activated in two
    # transposable row layouts (row d*64+j and row j*32+d)
    ctxp_raw_d = nc.dram_tensor("ctxp_raw", [CD, D * NPIX], BF16).ap()
    ctxp_dmaj_d = nc.dram_tensor("ctxp_dmaj", [D * CD, NPIX], BF16).ap()
    ctxp_jmaj_d = nc.dram_tensor("ctxp_jmaj", [D * CD, NPIX], BF16).ap()

    dbg = {}
    if DEBUG:
        for nm, shape, dt in [
            ("xp_dbg", [CH, NPIX], F32),
            ("ctxpf_dbg", [CD, D * NPIX], BF16),
            ("qt_dbg", [NPIX, 512], BF16),
            ("sim_dbg", [NPIX, 256], F32),
            ("a_dbg", [NPIX, 256], BF16),
            ("ctxa_dbg", [NPIX, 512], BF16),
            ("out1_dbg", [CH, NPIX], F32),
            ("y1_dbg", [CH, NPIX], F32),
            ("y2_dbg", [CH, NPIX], F32),
        ]:
            dbg[nm] = nc.dram_tensor(nm, shape, dt, kind="ExternalOutput").ap()

    with tile.TileContext(nc) as tc:
        from contextlib import ExitStack

        es = ExitStack()
        persist = es.enter_context(tc.tile_pool(name="persist", bufs=1))
        wpool = es.enter_context(tc.tile_pool(name="wpool", bufs=1))
        wstream = es.enter_context(tc.tile_pool(name="wstream", bufs=1))
        stage = es.enter_context(tc.tile_pool(name="stage", bufs=3))
        small = es.enter_context(tc.tile_pool(name="small", bufs=2))
        attn = es.enter_context(tc.tile_pool(name="attn", bufs=2))
        psum = es.enter_context(tc.tile_pool(name="psum", bufs=6, space="PSUM"))
        spsum = es.enter_context(tc.tile_pool(name="spsum", bufs=2, space="PSUM"))

        # ---------------- load persistent weights ----------------
        _wn = [0]
        def load_w(pool, src, shape, dt=F32, tag=None):
            _wn[0] += 1
            t = pool.tile(shape, dt, tag=tag or "", name=f"w{_wn[0]}")
            nc.sync.dma_start(out=t[:], in_=src)
            return t

        win_sb = [[load_w(wpool, w_in_t[k, m], [128, 128], BF16) for m in range(NT)]
                  for k in range(NT)]
        wqkt_sb = [load_w(wpool, wqkt_d[k], [128, 512], BF16) for k in range(NT)]
        wctx_sb = load_w(wpool, wctx_t, [CD, CD], BF16)
        wovt_sb = [[load_w(wpool, wovt_d[k, m], [128, 128], BF16) for m in range(NT)]
                   for k in range(4)]
        gsel_sb = [[load_w(wpool, gsel_d[k, m], [128, 128]) for m in range(NT)]
                   for k in range(NT)]
        g2c_sb = load_w(wpool, g2c_d, [CD, CD])
        bin_sb = [load_w(wpool, b_in[m * 128:(m + 1) * 128], [128, 1]) for m in range(NT)]

        def load_vec(src):
            return [load_w(wpool, src[m * 128:(m + 1) * 128], [128, 1]) for m in range(NT)]

        gin_g_sb, gin_b_sb = load_vec(gin_g), load_vec(gin_b)
        g1_g_sb, g1_b_sb = load_vec(g1_g), load_vec(g1_b)
        g2_g_sb, g2_b_sb = load_vec(g2_g), load_vec(g2_b)
        gctx_g_sb = load_w(wpool, gctx_g, [CD, 1])
        gctx_b_sb = load_w(wpool, gctx_b, [CD, 1])

        eps_sb = wpool.tile([128, 1], F32)
        nc.vector.memset(eps_sb[:], EPS)

        posr_dj = wpool.tile([128, 2048], BF16)
        nc.sync.dma_start(out=posr_dj[:],
                          in_=pos_dj.partition_broadcast(128).squeeze(1))
        posr_jd = wpool.tile([128, 2048], BF16)
        nc.sync.dma_start(out=posr_jd[:],
                          in_=pos_jd.partition_broadcast(128).squeeze(1))

        # =========== GN stats helper (320-channel, 8 groups of 40) ===========
        def gn_affine_320(raw_tiles, gamma_sb, beta_sb, name):
            """raw_tiles: 3x [128,1024] f32 SBUF. Returns (s, t) per-tile [128,1]."""
            mv2 = []
            for t in range(NT):
                st = small.tile([128, 2, 6], F32, tag=f"{name}_st{t}", name=f"{name}_st{t}")
                for i in range(2):
                    nc.vector.bn_stats(out=st[:, i, :], in_=raw_tiles[t][:, i * 512:(i + 1) * 512])
                mv = small.tile([128, 2], F32, tag=f"{name}_mv{t}", name=f"{name}_mv{t}")
                nc.vector.bn_aggr(out=mv[:], in_=st[:])
                m2 = small.tile([128, 2], F32, tag=f"{name}_mv2{t}", name=f"{name}_m2{t}")
                nc.vector.tensor_copy(m2[:, 0:1], mv[:, 0:1])
                nc.vector.tensor_mul(m2[:, 1:2], mv[:, 0:1], mv[:, 0:1])
                nc.vector.tensor_add(m2[:, 1:2], m2[:, 1:2], mv[:, 1:2])
                mv2.append(m2)
            s_t = []
            for m in range(NT):
                gp = spsum.tile([128, 2], F32, tag="gnps")
                for k in range(NT):
                    nc.tensor.matmul(gp[:], gsel_sb[k][m][:], mv2[k][:],
                                     start=(k == 0), stop=(k == NT - 1))
                gs = small.tile([128, 2], F32, tag=f"{name}_gs{m}")
                nc.vector.tensor_copy(gs[:], gp[:])
                s = small.tile([128, 1], F32, tag=f"{name}_s{m}")
                tt = small.tile([128, 1], F32, tag=f"{name}_t{m}")
                vt = small.tile([128, 1], F32, tag=f"{name}_v")
                nc.vector.tensor_mul(vt[:], gs[:, 0:1], gs[:, 0:1])
                nc.vector.tensor_sub(vt[:], gs[:, 1:2], vt[:])
                nc.scalar.activation(out=vt[:], in_=vt[:], func=AF.Sqrt, bias=eps_sb[:, 0:1])
                nc.vector.reciprocal(out=vt[:], in_=vt[:])
                nc.vector.tensor_mul(s[:], gamma_sb[m][:], vt[:])
                nc.vector.tensor_mul(tt[:], gs[:, 0:1], s[:])
                nc.vector.tensor_sub(tt[:], beta_sb[m][:], tt[:])
                s_t.append((s, tt))
            return s_t

        # ---------------- phase 1: x load + proj_in + gn_in + silu ----------------
        x_sb = [persist.tile([128, NPIX], F32, name=f"x_{t}") for t in range(NT)]
        xbf_sb = [persist.tile([128, NPIX], BF16, name=f"xb_{t}") for t in range(NT)]
        for t in range(NT):
            nc.sync.dma_start(out=x_sb[t][:], in_=x_d[t * 128:(t + 1) * 128, :])
            nc.sync.dma_start(out=xbf_sb[t][:], in_=xbf_d[t * 128:(t + 1) * 128, :])

        xp = [persist.tile([128, NPIX], F32, name=f"xp_{t}") for t in range(NT)]
        xp_bf = [persist.tile([128, NPIX], BF16, name=f"xpb_{t}") for t in range(NT)]
        for m in range(NT):
            for n in range(2):
                ps = psum.tile([128, 512], F32, tag="mm")
                for k in range(NT):
                    nc.tensor.matmul(ps[:], win_sb[k][m][:], xbf_sb[k][:, n * 512:(n + 1) * 512],
                                     start=(k == 0), stop=(k == NT - 1))
                nc.scalar.activation(out=xp[m][:, n * 512:(n + 1) * 512], in_=ps[:],
                                     func=AF.Identity, bias=bin_sb[m][:, 0:1])
        st_in = gn_affine_320(xp, gin_g_sb, gin_b_sb, "gin")
        for m in range(NT):
            s, t = st_in[m]
            nc.scalar.activation(out=xp_bf[m][:], in_=xp[m][:], func=AF.Silu,
                                 bias=t[:, 0:1], scale=s[:, 0:1])
        if DEBUG:
            for m in range(NT):
                hi = min(128, CH - m * 128)
                nc.sync.dma_start(out=dbg["xp_dbg"][m * 128:m * 128 + hi, :], in_=xp[m][:hi, :])  # pre-silu tap unused

        # ---------------- phase 2: ctx proj + gn stats; raw bf16 -> DRAM ----------------
        cst = persist.tile([CD, 64, 6], F32)
        for c in range(64):
            cin = stage.tile([CD, 512], BF16, tag="ctxin")
            nc.sync.dma_start(out=cin[:], in_=ctx_d[:, c * 512:(c + 1) * 512])
            ps = psum.tile([CD, 512], F32, tag="mm")
            nc.tensor.matmul(ps[:], wctx_sb[:], cin[:], start=True, stop=True)
            crawbf = stage.tile([CD, 512], BF16, tag="ctxev")
            nc.vector.tensor_copy(crawbf[:], ps[:])
            nc.vector.bn_stats(out=cst[:, c, :], in_=crawbf[:])
            nc.sync.dma_start(out=ctxp_raw_d[:, c * 512:(c + 1) * 512], in_=crawbf[:])
        cmv = small.tile([CD, 2], F32)
        nc.vector.bn_aggr(out=cmv[:], in_=cst[:])
        cmv2 = small.tile([CD, 2], F32)
        nc.vector.tensor_copy(cmv2[:, 0:1], cmv[:, 0:1])
        nc.vector.tensor_mul(cmv2[:, 1:2], cmv[:, 0:1], cmv[:, 0:1])
        nc.vector.tensor_add(cmv2[:, 1:2], cmv2[:, 1:2], cmv[:, 1:2])
        cgp = spsum.tile([CD, 2], F32, tag="gnps")
        nc.tensor.matmul(cgp[:], g2c_sb[:], cmv2[:], start=True, stop=True)
        cgs = small.tile([CD, 2], F32)
        nc.vector.tensor_copy(cgs[:], cgp[:])
        cs = small.tile([CD, 1], F32)
        ct = small.tile([CD, 1], F32)
        cv = small.tile([CD, 1], F32)
        nc.vector.tensor_mul(cv[:], cgs[:, 0:1], cgs[:, 0:1])
        nc.vector.tensor_sub(cv[:], cgs[:, 1:2], cv[:])
        nc.scalar.activation(out=cv[:], in_=cv[:], func=AF.Sqrt, bias=eps_sb[:CD, 0:1])
        nc.vector.reciprocal(out=cv[:], in_=cv[:])
        nc.vector.tensor_mul(cs[:], gctx_g_sb[:], cv[:])
        nc.vector.tensor_mul(ct[:], cgs[:, 0:1], cs[:])
        nc.vector.tensor_sub(ct[:], gctx_b_sb[:], ct[:])
        # apply relu(x*s+t) in chunks; scatter rows into both DRAM layouts
        for c in range(8):
            d0 = c * 4
            sl = slice(c * 4096, (c + 1) * 4096)
            ca = stage.tile([CD, 4, NPIX], BF16, tag="ctxap")
            nc.sync.dma_start(out=ca[:], in_=ctxp_raw_d[:, sl])
            nc.scalar.activation(out=ca[:], in_=ca[:], func=AF.Relu,
                                 bias=ct[:, 0:1], scale=cs[:, 0:1])
            odm = bass.AP(tensor=ctxp_dmaj_d.tensor, offset=d0 * CD * NPIX,
                          ap=[[NPIX, CD], [CD * NPIX, 4], [1, NPIX]])
            nc.sync.dma_start(out=odm, in_=ca[:])
            ojm = bass.AP(tensor=ctxp_jmaj_d.tensor, offset=d0 * NPIX,
                          ap=[[D * NPIX, CD], [NPIX, 4], [1, NPIX]])
            nc.sync.dma_start(out=ojm, in_=ca[:])
        if DEBUG:
            for c in range(8):
                d0 = c * 4
                cb = stage.tile([CD, 4, NPIX], BF16, tag="ctxdbg")
                idm = bass.AP(tensor=ctxp_dmaj_d.tensor, offset=d0 * CD * NPIX,
                              ap=[[NPIX, CD], [CD * NPIX, 4], [1, NPIX]])
                nc.sync.dma_start(out=cb[:], in_=idm)
                nc.sync.dma_start(out=dbg["ctxpf_dbg"][:, c * 4096:(c + 1) * 4096], in_=cb[:])

        # ---------------- phase 3: qt (pixel-major q-tilde) ----------------
        qt = [persist.tile([128, 512], BF16, name=f"qt_{p}") for p in range(8)]
        for p in range(8):
            ps = psum.tile([128, 512], F32, tag="mm")
            for k in range(NT):
                nc.tensor.matmul(ps[:], xp_bf[k][:, p * 128:(p + 1) * 128], wqkt_sb[k][:],
                                 start=(k == 0), stop=(k == NT - 1))
            nc.scalar.activation(out=qt[p][:], in_=ps[:], func=AF.Copy)
            if DEBUG:
                nc.sync.dma_start(out=dbg["qt_dbg"][p * 128:(p + 1) * 128, :], in_=qt[p][:])

        # ---------------- phase 4: attention per pixel-tile ----------------
        ctxa_bf = [persist.tile([128, 512], BF16, name=f"cxb_{p}") for p in range(8)]
        for p in range(8):
            tdj = attn.tile([128, 32, 64], BF16, tag="tdj")
            tjd = attn.tile([128, 64, 32], BF16, tag="tjd")
            nc.sync.dma_start_transpose(
                out=tdj[:].rearrange("p d j -> p (d j)"),
                in_=ctxp_dmaj_d[:, p * 128:(p + 1) * 128])
            nc.sync.dma_start_transpose(
                out=tjd[:].rearrange("p j d -> p (j d)"),
                in_=ctxp_jmaj_d[:, p * 128:(p + 1) * 128])
            nc.vector.tensor_add(
                tdj[:], tdj[:], posr_dj[:].rearrange("p (d j) -> p d j", j=64))
            nc.vector.tensor_add(
                tjd[:], tjd[:], posr_jd[:].rearrange("p (j d) -> p j d", d=32))

            sim = attn.tile([128, 8, 32], F32, tag="sim")
            for h in range(4):
                prod = attn.tile([128, 4096], BF16, tag="prod")
                in0 = _bcast(tdj[:], 1, 2)
                in1 = _bcast(
                    qt[p][:, h * 128:(h + 1) * 128].rearrange("p (n j) -> p n j", j=64),
                    2, 32)
                nc.vector.tensor_tensor(out=prod[:], in0=in0, in1=in1, op=ALU.mult)
                nc.vector.tensor_reduce(
                    out=sim[:, h * 2:(h + 1) * 2, :],
                    in_=prod[:].rearrange("p (nd j) -> p nd j", j=64),
                    axis=AX.X, op=ALU.add)
            if DEBUG:
                nc.sync.dma_start(out=dbg["sim_dbg"][p * 128:(p + 1) * 128, :],
                                  in_=sim[:].rearrange("p n d -> p (n d)"))
            # softmax over d
            mx = small.tile([128, 8], F32, tag="mx")
            nc.vector.tensor_reduce(out=mx[:], in_=sim[:], axis=AX.X, op=ALU.max, negate=True)
            for n in range(HN):
                nc.scalar.activation(out=sim[:, n, :], in_=sim[:, n, :], func=AF.Exp,
                                     bias=mx[:, n:n + 1])
            sm = small.tile([128, 8], F32, tag="sm")
            nc.vector.tensor_reduce(out=sm[:], in_=sim[:], axis=AX.X, op=ALU.add)
            nc.vector.reciprocal(out=sm[:], in_=sm[:])
            abf = attn.tile([128, 8, 32], BF16, tag="abf")
            nc.vector.tensor_tensor(
                out=abf[:], in0=sim[:], in1=_bcast(sm[:], 2, 32), op=ALU.mult)
            if DEBUG:
                nc.sync.dma_start(out=dbg["a_dbg"][p * 128:(p + 1) * 128, :],
                                  in_=abf[:].rearrange("p n d -> p (n d)"))
            # ctxa
            cxf = attn.tile([128, 512], F32, tag="cxf")
            for h in range(4):
                prod = attn.tile([128, 4096], BF16, tag="prod")
                in0 = _bcast(tjd[:], 1, 2)
                in1 = _bcast(abf[:, h * 2:(h + 1) * 2, :], 2, 64)
                nc.vector.tensor_tensor(out=prod[:], in0=in0, in1=in1, op=ALU.mult)
                nc.vector.tensor_reduce(
                    out=cxf[:, h * 128:(h + 1) * 128],
                    in_=prod[:].rearrange("p (nj d) -> p nj d", d=32),
                    axis=AX.X, op=ALU.add)
            nc.gpsimd.tensor_copy(ctxa_bf[p][:], cxf[:])
            if DEBUG:
                nc.sync.dma_start(out=dbg["ctxa_dbg"][p * 128:(p + 1) * 128, :], in_=ctxa_bf[p][:])

        # ---------------- phase 5: transpose ctxa to ch-major ----------------
        ctxa_ch = [persist.tile([128, NPIX], BF16, name=f"cxc_{k}") for k in range(4)]
        for kt in range(4):
            for p in range(8):
                nc.sync.dma_start_transpose(
                    out=ctxa_ch[kt][:, p * 128:(p + 1) * 128],
                    in_=ctxa_bf[p][:, kt * 128:(kt + 1) * 128])

        # ---------------- phase 6: Wov proj -> out1; gn1+relu -> y1pad ----------------
        out1 = [persist.tile([128, NPIX], F32, tag=f"out1_{m}", name=f"out1_{m}") for m in range(NT)]
        for m in range(NT):
            for n in range(2):
                ps = psum.tile([128, 512], F32, tag="mm")
                for k in range(4):
                    nc.tensor.matmul(ps[:], wovt_sb[k][m][:],
                                     ctxa_ch[k][:, n * 512:(n + 1) * 512],
                                     start=(k == 0), stop=(k == 3))
                nc.scalar.activation(out=out1[m][:, n * 512:(n + 1) * 512], in_=ps[:], func=AF.Copy)
        if DEBUG:
            for m in range(NT):
                hi = min(128, CH - m * 128)
                nc.sync.dma_start(out=dbg["out1_dbg"][m * 128:m * 128 + hi, :], in_=out1[m][:hi, :])

        st1 = gn_affine_320(out1, g1_g_sb, g1_b_sb, "gn1")
        pad1 = [persist.tile([128, 34, 34], BF16, tag=f"pad1_{m}", name=f"pad1_{m}") for m in range(NT)]
        for m in range(NT):
            nc.vector.memset(pad1[m][:], 0.0)
            s, t = st1[m]
            nc.scalar.activation(out=pad1[m][:, 1:33, 1:33],
                                 in_=out1[m][:].rearrange("p (h w) -> p h w", w=32),
                                 func=AF.Relu, bias=t[:, 0:1], scale=s[:, 0:1])
        if DEBUG:
            for m in range(NT):
                hi = min(128, CH - m * 128)
                nc.sync.dma_start(
                    out=dbg["y1_dbg"][m * 128:m * 128 + hi, :],
                    in_=pad1[m][:hi, 1:33, 1:33].rearrange("p h w -> p (h w)"))

        # ---------------- conv helper ----------------
        def conv3x3(w_d, src_pad, name):
            """9-tap conv; returns out tiles 3x[128,1024] f32."""
            cw = [[wstream.tile([128, 9, 128], BF16, tag=f"cw_{k}_{m}",
                                name=f"{name}w_{k}_{m}") for m in range(NT)]
                  for k in range(NT)]
            for k in range(NT):
                for m in range(NT):
                    nc.sync.dma_start(out=cw[k][m][:], in_=w_d[k, m])
            out = [persist.tile([128, NPIX], F32, tag=f"out1_{m}", name=f"cv_{m}") for m in range(NT)]
            for m in range(NT):
                for n in range(2):
                    r0 = n * 16
                    ps = psum.tile([128, 512], F32, tag="mm")
                    first = True
                    for tap in range(9):
                        dy, dx = tap // 3, tap % 3
                        for k in range(NT):
                            nc.tensor.matmul(
                                ps[:], cw[k][m][:, tap, :],
                                src_pad[k][:, r0 + dy:r0 + dy + 16, dx:dx + 32],
                                start=first, stop=(tap == 8 and k == NT - 1))
                            first = False
                    nc.scalar.activation(out=out[m][:, n * 512:(n + 1) * 512], in_=ps[:],
                                         func=AF.Copy)
            return out

        y2 = conv3x3(c1_d, pad1, "c1")
        st2 = gn_affine_320(y2, g2_g_sb, g2_b_sb, "gn2")
        pad2 = [persist.tile([128, 34, 34], BF16, tag=f"pad1_{m}", name=f"pad2_{m}") for m in range(NT)]
        for m in range(NT):
            nc.vector.memset(pad2[m][:], 0.0)
            s, t = st2[m]
            nc.scalar.activation(out=pad2[m][:, 1:33, 1:33],
                                 in_=y2[m][:].rearrange("p (h w) -> p h w", w=32),
                                 func=AF.Relu, bias=t[:, 0:1], scale=s[:, 0:1])
        if DEBUG:
            for m in range(NT):
                hi = min(128, CH - m * 128)
                nc.sync.dma_start(
                    out=dbg["y2_dbg"][m * 128:m * 128 + hi, :],
                    in_=pad2[m][:hi, 1:33, 1:33].rearrange("p h w -> p (h w)"))

        # ---------------- conv2 + residual ----------------
        cw2 = [[wstream.tile([128, 9, 128], BF16, tag=f"cw_{k}_{m}",
                             name=f"c2w_{k}_{m}") for m in range(NT)]
               for k in range(NT)]
        for k in range(NT):
            for m in range(NT):
                nc.sync.dma_start(out=cw2[k][m][:], in_=c2_d[k, m])
        for m in range(NT):
            hi = min(128, CH - m * 128)
            for n in range(2):
                r0 = n * 16
                ps = psum.tile([128, 512], F32, tag="mm")
                first = True
                for tap in range(9):
                    dy, dx = tap // 3, tap % 3
                    for k in range(NT):
                        nc.tensor.matmul(
                            ps[:], cw2[k][m][:, tap, :],
                            pad2[k][:, r0 + dy:r0 + dy + 16, dx:dx + 32],
                            start=first, stop=(tap == 8 and k == NT - 1))
                        first = False
                fin = stage.tile([128, 512], F32, tag="fin")
                nc.vector.tensor_add(fin[:], ps[:], x_sb[m][:, n * 512:(n + 1) * 512])
                nc.sync.dma_start(out=y_d[m * 128:m * 128 + hi, n * 512:(n + 1) * 512],
                                  in_=fin[:hi, :])
        es.close()

    nc.compile()
    return nc


_PROG = None
_LAST_RESULTS = None
_LAST_EXEC_NS = None


def _get_prog():
    global _PROG
    if _PROG is None:
        _PROG = build_program()
    return _PROG


def _prep_host(inputs):
    """Precompute folded weights; returns the common (weight) part of in_map."""
    f32 = np.float32
    bf16 = ml_dtypes.bfloat16
    w_in = np.asarray(inputs["w_in"], f32)
    wq = np.asarray(inputs["wq"], f32)
    wk = np.asarray(inputs["wk"], f32)
    wv = np.asarray(inputs["wv"], f32)
    wout = np.asarray(inputs["w_attn_out"], f32)
    pos = np.asarray(inputs["pos_emb"], f32)
    scale = HD ** -0.5

    def pad_to(a, shape):
        out = np.zeros(shape, a.dtype)
        out[tuple(slice(0, s) for s in a.shape)] = a
        return out

    def tile_km(mat_t, kt, mt):  # mat_t: [K, M] -> [kt, mt, 128, 128]
        p = pad_to(mat_t, (kt * 128, mt * 128))
        return np.ascontiguousarray(
            p.reshape(kt, 128, mt, 128).transpose(0, 2, 1, 3))

    w_in_tiles = tile_km(w_in.T, NT, NT).astype(bf16)

    wqk = np.concatenate(
        [scale * (wk[n * HD:(n + 1) * HD, :].T @ wq[n * HD:(n + 1) * HD, :])
         for n in range(HN)], axis=0)          # [512, 320]
    wqkt = pad_to(wqk.T, (NT * 128, 512)).reshape(NT, 128, 512).astype(bf16)

    wov = np.concatenate(
        [wout[:, n * HD:(n + 1) * HD] @ wv[n * HD:(n + 1) * HD, :]
         for n in range(HN)], axis=1)          # [320, 512]
    wov_tiles = tile_km(wov.T, 4, NT).astype(bf16)

    def conv_tiles(w):  # [o, i, 3, 3] -> [kt, mt, 128, 9, 128] (tap-interleaved lhsT)
        taps = np.stack([tile_km(np.ascontiguousarray(w[:, :, t // 3, t % 3].T), NT, NT)
                         for t in range(9)], axis=0)  # [9, kt, mt, 128, 128]
        return np.ascontiguousarray(taps.transpose(1, 2, 3, 0, 4)).astype(bf16)

    gsel = np.zeros((CH, CH), f32)
    for g in range(8):
        gsel[g * 40:(g + 1) * 40, g * 40:(g + 1) * 40] = 1.0 / 40
    g2ctx = np.zeros((CD, CD), f32)
    for g in range(8):
        g2ctx[g * 8:(g + 1) * 8, g * 8:(g + 1) * 8] = 1.0 / 8

    def col(v):
        return pad_to(np.asarray(v, f32).reshape(-1, 1), (384, 1))

    common = {
        "w_in_t": w_in_tiles,
        "b_in": col(inputs["b_in"]),
        "gin_g": col(inputs["gn_in_g"]), "gin_b": col(inputs["gn_in_b"]),
        "wctx_t": np.ascontiguousarray(np.asarray(inputs["w_ctx"], f32).T).astype(bf16),
        "gctx_g": np.asarray(inputs["gn_ctx_g"], f32).reshape(CD, 1),
        "gctx_b": np.asarray(inputs["gn_ctx_b"], f32).reshape(CD, 1),
        "wqkt": wqkt,
        "pos_dj": pos.reshape(1, 2048).astype(bf16),
        "pos_jd": np.ascontiguousarray(pos.T).reshape(1, 2048).astype(bf16),
        "wovt": wov_tiles,
        "g1_g": col(inputs["gn1_g"]), "g1_b": col(inputs["gn1_b"]),
        "g2_g": col(inputs["gn2_g"]), "g2_b": col(inputs["gn2_b"]),
        "conv1_t": conv_tiles(np.asarray(inputs["conv1_w"], f32)),
        "conv2_t": conv_tiles(np.asarray(inputs["conv2_w"], f32)),
        "gsel": tile_km(gsel, NT, NT),
        "g2ctx": g2ctx,
    }
    return common


def kernel(**inputs):
    nc = _get_prog()
    common = _prep_host(inputs)
    x = np.asarray(inputs["x"], np.float32)      # [6, 320, 32, 32]
    ctx = np.asarray(inputs["context"], np.float32)  # [6, 64, 32, 32, 32]
    b = x.shape[0]
    in_maps = []
    for core in range(8):
        s = core if core < b else core - b
        m = dict(common)
        xs = np.zeros((384, NPIX), np.float32)
        xs[:CH] = x[s].reshape(CH, NPIX)
        m["x"] = xs
        m["x_bf"] = xs.astype(ml_dtypes.bfloat16)
        m["ctxin"] = np.ascontiguousarray(
            ctx[s].reshape(CD, D * NPIX)).astype(ml_dtypes.bfloat16)
        in_maps.append(m)
    trace = bool(int(os.environ.get("DT_TRACE", "0")))
    kw = {}
    if trace:
        import sys
        import types
        try:
            import antenv.axon_hooks  # noqa: F401
        except ImportError:
            from trn_agent_boot.trn_boot import _ntff_profile_via_ctypes
            m = types.ModuleType("antenv.axon_hooks")
            _h = _ntff_profile_via_ctypes("/opt/axon/libaxon_pjrt.so")
            m.get_axon_ntff_profile_hook = lambda: _h
            sys.modules["antenv.axon_hooks"] = m
        kw = dict(trace=True, tmpdir=os.environ.get("DT_TRACE_DIR") or None)
    res = run_bass_kernel_spmd(nc, in_maps, list(range(8)), **kw)
    global _LAST_RESULTS, _LAST_EXEC_NS
    _LAST_RESULTS = res.results
    _LAST_EXEC_NS = res.exec_time_ns
    if trace:
        print(f"HW exec time: {res.exec_time_ns} ns")
    out = np.stack([res.results[s]["y"] for s in range(b)], axis=0)
    return out.reshape(b, CH, 32, 32).astype(np.float32)


if __name__ == "__main__":
    pass
